# revision 9
# baseline (speedup 1.0000x reference)
"""Trainium2 Bass kernel for suffix-softmax attention visualization.

Computes, for hidden_states [S, B, H], W [H, 1], b [1]:
    s[t, b]   = sum_h hidden_states[t, b, h] * W[h, 0] + b[0]
    out[t, b] = exp(s[t, b]) / sum_{t' >= t} exp(s[t', b])     (suffix softmax)
returned as [S, B, 1] f32.

The softmax ratio is shift-invariant, so the scalar bias b cancels exactly
and is not needed on device.

Sharding: data-parallel over the batch axis — 8 NeuronCores, 8 batch
columns each. The kernel is a pure HBM stream: 64 MB f32 per core.

v2 design (vs the earlier SWDGE fp16-cast pipeline):
  - The input streams as RAW f32 over the Sync HWDGE ring. The previous
    SWDGE cast-DMA path hit the known SDMA-engine-15 slowdown (SWDGE
    descriptor rings share engine 15's SBUF AXI port), which made every
    128-partition block wait on one ~20%-slower engine and left a ~35 us
    single-engine tail. HWDGE has no SBUF descriptor ring and is immune;
    all 16 SDMA engines then run uniformly (~16 KB/partition descriptors
    at ~25 GB/s/engine => ~5.1 us per [128 seq, 8 b, 512 h] block).
  - With f32 data everything runs 1x. The dot product is split three
    ways: `dve_cols` columns run as fused scalar_tensor_tensor(+accum)
    on the DVE (~0.77 us/col; the backend rejects STT on Pool); the
    remaining columns have their products materialized by the
    now-otherwise-idle Pool engine (tensor_tensor f32 -> fp16, ~0.9 us
    per 2-col instruction) and are h-reduced by ACT copy-accumulate
    (~1.0 us/col). Per-block engine loads (~3.4 us DVE, ~4.3 us ACT,
    ~2.4 us Pool) sit below the ~5.1 us DMA slot, so the stream paces
    even if compute throttles ~15-25%.
  - blocks stream in REVERSE seq order (suffix accumulates forward);
    the suffix state lives in one PSUM tile R [128, 8]:
    matmul-accumulating lower-triangular ones gives R + within-block
    suffix-scan (the divisor), then strictly-upper ones turn it into
    the next running total, broadcast across partitions, on the
    otherwise-idle PE.
  - the finalize (reciprocal on DVE, multiply on Pool) is deferred one
    block so nothing waits on the exp -> matmul chain;
  - outputs collect in SBUF and DMA out in 4-block chunks on the
    Scalar HWDGE ring (separate FIFO from the input stream).
"""

import numpy as np

import concourse.bacc as bacc
import concourse.mybir as mybir
import concourse.tile as tile
from concourse import bass_utils

P = 128
S = 4096
B = 64
H = 512
N_CORES = 8
BC = B // N_CORES  # batch columns per core
NBLK = S // P


def build_program(hs_bufs=9, out_chunk=4, look=7, dve_cols=4, prod_dt="fp16", Bc=BC):
    """Build the per-core Bass program.

    Inputs : hs [S, Bc, H] f32, wb [128, H] f32 (W broadcast),
             tri [128, 128] f32 lower-triangular ones (suffix scan),
             triu [128, 128] f32 strictly-upper ones (running-total update).
    Output : out [S, Bc] f32.

    dve_cols: columns computed as fused STT on DVE; the remaining
    Bc - dve_cols columns go Pool-multiply -> ACT copy-accumulate.
    """
    assert S % P == 0
    nblk = S // P
    assert nblk % out_chunk == 0
    assert hs_bufs >= look + 2

    nc = bacc.Bacc("TRN2", target_bir_lowering=False, debug=False)
    hs = nc.dram_tensor("hs", [S, Bc, H], mybir.dt.float32, kind="ExternalInput")
    wb = nc.dram_tensor("wb", [P, H], mybir.dt.float32, kind="ExternalInput")
    tri = nc.dram_tensor("tri", [P, P], mybir.dt.float32, kind="ExternalInput")
    triu = nc.dram_tensor("triu", [P, P], mybir.dt.float32, kind="ExternalInput")
    out = nc.dram_tensor("out", [S, Bc], mybir.dt.float32, kind="ExternalOutput")

    # Processing order: last seq block first (suffix accumulates forward).
    order = list(range(nblk - 1, -1, -1))
    # cols-per-DMA-chunk by processing index: small chunks at the ends so
    # compute starts early (ramp) and the last block's columns finalize as
    # they land (drain).
    split_plan = {0: 2, 1: 4, nblk - 1: 4}

    with tile.TileContext(nc) as tc:
        with (
            tc.tile_pool(name="hsp", bufs=hs_bufs) as hsp,
            tc.tile_pool(name="consts", bufs=1) as consts,
            tc.tile_pool(name="work", bufs=1) as work,
            tc.tile_pool(name="sp", bufs=4) as sp,
            tc.tile_pool(name="ep", bufs=4) as ep,
            tc.tile_pool(name="lsep", bufs=3) as lsep,
            tc.tile_pool(name="prodp", bufs=3) as prodp,
            tc.tile_pool(name="psum", bufs=1, space="PSUM") as psum,
        ):
            hs_ap = hs.ap()
            hs_tiles = {}

            def issue_dma(idx):
                j = order[idx]
                hst = hsp.tile([P, Bc, H], mybir.dt.float32)
                rows = hs_ap[j * P : (j + 1) * P, :, :]
                qb = min(split_plan.get(idx, Bc), Bc)
                for q in range(0, Bc, qb):
                    nc.sync.dma_start(
                        out=hst[:, q : q + qb, :], in_=rows[:, q : q + qb, :]
                    )
                hs_tiles[j] = hst

            wb_t = consts.tile([P, H], mybir.dt.float32)
            nc.scalar.dma_start(out=wb_t, in_=wb.ap())
            wbr_t = consts.tile([P, 2 * H], mybir.dt.float32)
            nc.scalar.dma_start(out=wbr_t[:, :H], in_=wb.ap())
            nc.scalar.dma_start(out=wbr_t[:, H:], in_=wb.ap())
            tri_t = consts.tile([P, P], mybir.dt.float32)
            nc.scalar.dma_start(out=tri_t, in_=tri.ap())
            triu_t = consts.tile([P, P], mybir.dt.float32)
            nc.scalar.dma_start(out=triu_t, in_=triu.ap())

            for idx in range(look):
                issue_dma(idx)

            # Separate per-engine throwaway out-tiles: sharing one creates a
            # false WAW dependency that serializes the engines.
            dummy_v = work.tile([P, H], mybir.dt.float32)
            dummy_act = work.tile([P, H], mybir.dt.float16)
            sel_buf = work.tile([P, nblk * Bc], mybir.dt.float32)
            r_ps = psum.tile([P, Bc], mybir.dt.float32)

            out_ap = out.ap().rearrange("(blk p) b -> p blk b", p=P)

            def emit_finalize(j, s_t, e_t):
                lo = j * Bc
                # sel = e * (1/(R + scan)): reciprocal on DVE (ACT's is
                # banned for accuracy), multiply on Pool.
                rec_t = lsep.tile([P, Bc], mybir.dt.float32)
                nc.vector.reciprocal(rec_t, r_ps)
                nc.gpsimd.tensor_mul(sel_buf[:, lo : lo + Bc], e_t, rec_t)
                if j == 1:
                    # Flush blocks 1..out_chunk-1 early so the very last DMA
                    # (after block 0's finalize) is a single small block.
                    sel_ap = sel_buf[:, Bc : out_chunk * Bc].rearrange(
                        "p (blk b) -> p blk b", b=Bc
                    )
                    nc.scalar.dma_start(out=out_ap[:, 1:out_chunk, :], in_=sel_ap)
                elif j == 0:
                    sel_ap = sel_buf[:, 0:Bc].rearrange(
                        "p (blk b) -> p blk b", b=Bc
                    )
                    nc.scalar.dma_start(out=out_ap[:, 0:1, :], in_=sel_ap)
                elif j % out_chunk == 0:
                    sel_ap = sel_buf[:, lo : lo + out_chunk * Bc].rearrange(
                        "p (blk b) -> p blk b", b=Bc
                    )
                    nc.scalar.dma_start(
                        out=out_ap[:, j : j + out_chunk, :], in_=sel_ap
                    )

            pending = None  # (j, s_t, e_t) awaiting its deferred finalize
            for idx, j in enumerate(order):
                hst = hs_tiles[j]
                s_t = sp.tile([P, Bc], mybir.dt.float32)
                e_t = ep.tile([P, Bc], mybir.dt.float32)

                # Dot product, three-way split: `dve_cols` fused STT columns
                # on the DVE; the rest multiplied on Pool (f32 -> fp16
                # products, halving the write/read port traffic) and
                # h-reduced by ACT copy-accumulate.
                ac = Bc - dve_cols
                if ac > 0:
                    pdt = (
                        mybir.dt.float16
                        if prod_dt == "fp16"
                        else mybir.dt.float32
                    )
                    prod_t = prodp.tile([P, ac, H], pdt)
                    # Two columns per Pool instruction to amortize dispatch.
                    for q in range(0, ac, 2):
                        qe = min(q + 2, ac)
                        nc.gpsimd.tensor_tensor(
                            prod_t[:, q:qe, :].rearrange("p b h -> p (b h)"),
                            hst[:, dve_cols + q : dve_cols + qe, :].rearrange(
                                "p b h -> p (b h)"
                            ),
                            wbr_t[:, : (qe - q) * H],
                            op=mybir.AluOpType.mult,
                        )
                for b in range(dve_cols):
                    nc.vector.scalar_tensor_tensor(
                        out=dummy_v,
                        in0=hst[:, b, :],
                        scalar=1.0,
                        in1=wb_t,
                        op0=mybir.AluOpType.mult,
                        op1=mybir.AluOpType.mult,
                        accum_out=s_t[:, b : b + 1],
                    )
                for q in range(ac):
                    nc.scalar.activation(
                        dummy_act,
                        prod_t[:, q, :],
                        mybir.ActivationFunctionType.Copy,
                        accum_out=s_t[:, dve_cols + q : dve_cols + q + 1],
                    )

                # Deferred finalize of the previous block: its R+scan divisor
                # has been sitting ready in PSUM, so nothing waits on the
                # cross-engine chain.
                if pending is not None:
                    pj, ps, pe = pending
                    emit_finalize(pj, ps, pe)
                    # R <- R + total(prev block), broadcast on all partitions.
                    # Must run after the reciprocal's read of R.
                    nc.tensor.matmul(r_ps, triu_t, pe, start=False, stop=True)

                if idx + look < nblk:
                    issue_dma(idx + look)

                nc.scalar.activation(
                    e_t, s_t, mybir.ActivationFunctionType.Exp
                )
                # R + within-block suffix scan -> the divisor for block j.
                nc.tensor.matmul(r_ps, tri_t, e_t, start=(idx == 0), stop=True)
                pending = (j, s_t, e_t)

            pj, ps, pe = pending
            emit_finalize(pj, ps, pe)

    nc.compile()
    return nc


_PROGRAM = None


def _get_program():
    global _PROGRAM
    if _PROGRAM is None:
        _PROGRAM = build_program()
    return _PROGRAM


def make_in_maps(hidden_states, W):
    hidden_states = np.asarray(hidden_states, dtype=np.float32)
    W = np.asarray(W, dtype=np.float32)
    wb = np.ascontiguousarray(
        np.broadcast_to(W[:, 0][None, :], (P, H)).astype(np.float32)
    )
    tri = np.tril(np.ones((P, P), dtype=np.float32))
    triu = np.triu(np.ones((P, P), dtype=np.float32), 1)
    in_maps = []
    for c in range(N_CORES):
        hs_c = np.ascontiguousarray(hidden_states[:, c * BC : (c + 1) * BC, :])
        in_maps.append({"hs": hs_c, "wb": wb, "tri": tri, "triu": triu})
    return in_maps


def assemble_output(results):
    cols = [results[c]["out"] for c in range(N_CORES)]
    return np.concatenate(cols, axis=1)[..., None].astype(np.float32)


def kernel(hidden_states, W, b):
    nc = _get_program()
    in_maps = make_in_maps(hidden_states, W)
    res = bass_utils.run_bass_kernel_spmd(nc, in_maps, core_ids=list(range(N_CORES)))
    return assemble_output(res.results)
